# revision 1
# baseline (speedup 1.0000x reference)
"""Layer-pipelined Trainium2 kernel for the 2-layer ReLU-RNN discriminator.

  B=64, T=512, I=256, H=512, O=1
  layer l: h_t = relu(x_t @ W_ih^T + b_ih + b_hh + h_{t-1} @ W_hh^T)
  out = sigmoid(h1 @ W_fc^T + b_fc)

Topology: 4 core-pairs (c, c+4). Core c in 0..3 runs LAYER 0 for sample
block [16c, 16c+16); core c+4 runs LAYER 1 for the same block. The pair
exchanges hidden-state chunks via a pair-wise AllGather (only the lo half
of the output is consumed = L0's h0; verified behavior on this stack).

All cores run the IDENTICAL program; roles differ only in data:
  - wg (input-projection weights over [x(2 kchunks) | recv(4 kchunks)]):
      L0: [W_ih0^T | 0], L1: [0 | W_ih1^T]
  - wh: W_hh0^T / W_hh1^T;  bias: b0 / b1
  - ebias ("early bias", used for iterations j < LAG): L0: b0 (its chunks
      are real from j=0), L1: -1e9 so relu clamps h to exactly 0 until the
      first real h0 chunk arrives -> correct zero initial state.
  - xt: L0: x^T of its block; L1: zeros.

Within a core: 16 samples as TWO 8-sample groups (A/B) advanced in
lockstep; each W_hh k,m-block is loaded once per step and feeds both
groups' matmuls (measured ~77ns per shared-weight block vs ~2x for
separate loads). Epilogue: relu on ACT (group A) and DVE (group B) in
parallel. PSUM chunk tiles are prefilled by the input GEMM (bias via K=1
matmuls), so the per-step epilogue is a single relu, no adds.
"""

import numpy as np
import ml_dtypes

import concourse.bass as bass
import concourse.mybir as mybir
from concourse.tile import TileContext
from concourse.bass_utils import run_bass_kernel_spmd
from concourse.alu_op_type import AluOpType

F16 = np.float16
NCORES = 8
B, T, I, H, O = 64, 512, 256, 512, 1
PAIRS = 4
NW = 16            # samples per core
GW = 8             # samples per group (2 groups)
CH = 16            # steps per chunk
SE = 8             # send (collective) every SE chunks
LAG = SE + 8       # pipeline lag (iterations); slack hides AG latency
KH = H // 128      # 4
KX = I // 128      # 2
KG = KX + KH       # gemm contraction chunks
HC = KH * CH * GW  # h-tile cols per group = 512

_ctr = [0]


def _split_multi_waits(nc):
    """Walrus in this container rejects >1 sync-wait per instruction."""
    n_split = 0
    for f in nc.m.functions:
        for bb in f.blocks:
            out = []
            changed = False
            for inst in bb.instructions:
                si = inst.sync_info
                waits = list(si.on_wait) if si is not None and si.on_wait else []
                if len(waits) > 1:
                    changed = True
                    n_split += 1
                    for w in waits[:-1]:
                        _ctr[0] += 1
                        nop = mybir.InstNoOp(
                            name=f"waitnop-{_ctr[0]}", ins=[], outs=[]
                        )
                        nop.engine = inst.engine
                        nop.sync_info = mybir.SyncInfo(on_wait=[w], on_update=[])
                        out.append(nop)
                    inst.sync_info = mybir.SyncInfo(
                        on_wait=[waits[-1]],
                        on_update=list(si.on_update) if si.on_update else [],
                    )
                out.append(inst)
            if changed:
                bb.instructions = out
    return n_split


def build_nc(n_steps=T, debug_dumps=False, no_coll=False, kg=KG, no_fc=False,
             se=None):
    se = SE if se is None else se
    lag = se + 8
    nch = n_steps // CH
    niter = nch + lag
    nc = bass.Bass("TRN2", num_devices=NCORES)
    f32 = mybir.dt.float32
    bf = mybir.dt.float16

    xt_d = nc.dram_tensor("xt", [128, KX * 2 * n_steps * GW], bf,
                          kind="ExternalInput")
    wg_d = nc.dram_tensor("wg", [128, KG * H], bf, kind="ExternalInput")
    wh_d = nc.dram_tensor("wh", [128, KH * H], bf, kind="ExternalInput")
    bias_d = nc.dram_tensor("bias", [KH, 128], bf, kind="ExternalInput")
    ebias_d = nc.dram_tensor("ebias", [KH, 128], bf, kind="ExternalInput")
    wfc_d = nc.dram_tensor("wfc", [128, KH * 8], bf, kind="ExternalInput")
    ind_d = nc.dram_tensor("ind", [KH, KH * CH * GW], bf,
                           kind="ExternalInput")
    bfc_d = nc.dram_tensor("bfc", [1, 1], f32, kind="ExternalInput")
    y_d = nc.dram_tensor("y", [niter, CH * NW], f32, kind="ExternalOutput")
    if debug_dumps:
        dh_d = nc.dram_tensor("dh", [128, niter * 2 * HC], bf,
                              kind="ExternalOutput")

    with TileContext(nc) as tc:
        with (
            tc.tile_pool(name="wts", bufs=1) as p_w,
            tc.tile_pool(name="h", bufs=se + 2) as p_h,
            tc.tile_pool(name="recv", bufs=3 if se == 1 else 2) as p_recv,
            tc.tile_pool(name="y", bufs=2) as p_y,
            tc.tile_pool(name="ps", bufs=1, space="PSUM") as p_ps,
            tc.tile_pool(name="fc", bufs=2, space="PSUM") as p_fc,
            tc.tile_pool(name="dram", bufs=3, space="DRAM") as p_dram,
        ):
            # ---- load inputs ----
            xt_sb = p_w.tile([128, KX * 2 * n_steps * GW], bf, tag="xt")
            nc.sync.dma_start(xt_sb[:], xt_d[:])
            wg_sb = p_w.tile([128, KG * H], bf, tag="wg")
            nc.sync.dma_start(wg_sb[:], wg_d[:])
            wh_sb = p_w.tile([128, KH * H], bf, tag="wh")
            nc.sync.dma_start(wh_sb[:], wh_d[:])
            bias_sb = p_w.tile([KH, 128], bf, tag="bias")
            nc.sync.dma_start(bias_sb[:], bias_d[:])
            ebias_sb = p_w.tile([KH, 128], bf, tag="ebias")
            nc.sync.dma_start(ebias_sb[:], ebias_d[:])
            wfc_sb = p_w.tile([128, KH * 8], bf, tag="wfc")
            nc.sync.dma_start(wfc_sb[:], wfc_d[:])
            bfc_sb = p_w.tile([1, 1], f32, tag="bfc")
            nc.sync.dma_start(bfc_sb[:], bfc_d[:])
            ind_sb = p_w.tile([KH, KH * CH * GW], bf, tag="ind")
            nc.sync.dma_start(ind_sb[:], ind_d[:])
            recv_zero = p_w.tile([128, 2 * HC], bf, tag="rz")
            nc.vector.memset(recv_zero[:], 0.0)

            recv_tiles = []
            ps_tiles = {}   # j -> (psA, psB)
            h_tiles = {}    # (j, g) -> tile

            def mslice(ps, m, r=None):
                if r is None:
                    return ps[:, m * CH * GW:(m + 1) * CH * GW]
                return ps[:, m * CH * GW + r * GW:m * CH * GW + (r + 1) * GW]

            # ---- input GEMM: prefill psum for chunk j (both groups) ----
            # Returns a list of thunks (one matmul each) so the caller can
            # interleave them into the chain steps as PE filler work that
            # absorbs the relu-epilogue round-trip latency.
            def gemm_thunks(j):
                if j < lag:
                    rsb, q = recv_zero, 0
                else:
                    sidx, q = divmod(j - lag, se)
                    rsb = recv_tiles[sidx]
                bsb = ebias_sb if j < lag else bias_sb
                ps = [p_ps.tile([128, KH * CH * GW], f32, tag=f"ps{g}{j % 2}",
                                name=f"ps{g}_{j}") for g in range(2)]
                ps_tiles[j] = ps
                thunks = []
                for g in range(2):
                    thunks.append(lambda g=g: nc.tensor.matmul(
                        ps[g][:], bsb[:], ind_sb[:],
                        start=True, stop=False,
                    ))
                for k in range(kg):
                    for m in range(KH):
                        w = wg_sb[:, k * H + m * 128:k * H + (m + 1) * 128]
                        for g in range(2):
                            if k < KX:
                                off = ((k * 2 + g) * n_steps + (j % nch) * CH) * GW
                                mov = xt_sb[:, off:off + CH * GW]
                            else:
                                off = ((q * 2 + g) * HC
                                       + (k - KX) * CH * GW)
                                mov = rsb[:, off:off + CH * GW]
                            thunks.append(
                                lambda g=g, m=m, w=w, mov=mov, k=k, ps=ps:
                                nc.tensor.matmul(
                                    mslice(ps[g], m), w, mov,
                                    start=False, stop=(k == kg - 1),
                                ))
                return thunks

            # ---- one chain step (both groups) ----
            # PE order: for k: for m: [mm_A, mm_B] — same-weight mms
            # adjacent so the stationary load is paid once per (k, m).
            def step(j, r):
                ps = ps_tiles[j]
                if not (j == 0 and r == 0):
                    rp = r - 1 if r > 0 else CH - 1
                    for k in range(KH):
                        ksrc = []
                        for g in range(2):
                            hsrc = (h_tiles[(j, g)] if r > 0
                                    else h_tiles[(j - 1, g)])
                            ksrc.append(
                                hsrc[:, k * CH * GW + rp * GW:
                                     k * CH * GW + (rp + 1) * GW])
                        for m in range(KH):
                            w = wh_sb[:, k * H + m * 128:k * H + (m + 1) * 128]
                            for g in range(2):
                                nc.tensor.matmul(
                                    mslice(ps[g], m, r),
                                    w,
                                    ksrc[g],
                                    start=False, stop=(k == KH - 1),
                                )
                # epilogue: relu psum -> h (A on ACT, B on DVE)
                for g in range(2):
                    h3 = h_tiles[(j, g)][:].rearrange(
                        "p (k x) -> p k x", k=KH)[:, :, r * GW:(r + 1) * GW]
                    p3 = ps[g][:].rearrange(
                        "p (m x) -> p m x", m=KH)[:, :, r * GW:(r + 1) * GW]
                    if g == 0:
                        nc.scalar.activation(
                            h3, p3, mybir.ActivationFunctionType.Relu)
                    else:
                        nc.vector.tensor_scalar_max(h3, p3, 0.0)

            # NOTE on step() weight sharing: the PE stream per (k) is
            # [mm(g=0,m=0..3), mm(g=1,m=0..3)] — reordered below to pair
            # same-weight mms adjacently.

            def fc(j):
                ps = p_fc.tile([8, 2 * CH * GW], f32, tag=f"fc{j % 2}",
                               name=f"fc{j}")
                for g in range(2):
                    for k in range(KH):
                        nc.tensor.matmul(
                            ps[:, g * CH * GW:(g + 1) * CH * GW],
                            wfc_sb[:, k * 8:(k + 1) * 8],
                            h_tiles[(j, g)][:, k * CH * GW:(k + 1) * CH * GW],
                            start=(k == 0), stop=(k == KH - 1),
                        )
                ysb = p_y.tile([1, 2 * CH * GW], f32, tag="y", name=f"y{j}")
                nc.scalar.activation(
                    ysb[:], ps[0:1, :], mybir.ActivationFunctionType.Sigmoid,
                    bias=bfc_sb[0:1, 0:1],
                )
                nc.sync.dma_start(y_d[j:j + 1, :], ysb[:])

            def send(j):
                if (j + 1) % se != 0:
                    return
                sidx = j // se
                if sidx * se >= nch:
                    return  # tail sends carry only garbage chunks
                bi = p_dram.tile([128, se * 2 * HC], bf, tag="bi",
                                 name=f"bi{sidx}")
                for qq in range(se):
                    jq = j - se + 1 + qq
                    for g in range(2):
                        nc.gpsimd.dma_start(
                            bi[:, (qq * 2 + g) * HC:(qq * 2 + g + 1) * HC],
                            h_tiles[(jq, g)][:])
                rs = p_recv.tile([128, se * 2 * HC], bf, tag="recv",
                                 name=f"rs{sidx}")
                if no_coll:
                    nc.gpsimd.dma_start(rs[:], bi[:])
                else:
                    bo = p_dram.tile([256, se * 2 * HC], bf, tag="bo",
                                     name=f"bo{sidx}")
                    nc.gpsimd.collective_compute(
                        "AllGather",
                        mybir.AluOpType.bypass,
                        replica_groups=[[0, 4], [1, 5], [2, 6], [3, 7]],
                        ins=[bi.opt()],
                        outs=[bo.opt()],
                    )
                    nc.gpsimd.dma_start(rs[:], bo[0:128, :])
                recv_tiles.append(rs)

            # ---- main loop ----
            for t in gemm_thunks(0):
                t()
            for j in range(niter):
                for g in range(2):
                    h_tiles[(j, g)] = p_h.tile([128, HC], bf, tag=f"h{g}",
                                               name=f"h{g}_{j}")
                pend = gemm_thunks(j + 1) if j + 1 < niter else []
                per = -(-len(pend) // CH) if pend else 0
                for r in range(CH):
                    step(j, r)
                    for t in pend[r * per:(r + 1) * per]:
                        t()
                if not no_fc:
                    fc(j)
                send(j)
                if debug_dumps:
                    for g in range(2):
                        nc.sync.dma_start(
                            dh_d[:, (j * 2 + g) * HC:(j * 2 + g + 1) * HC],
                            h_tiles[(j, g)][:])

    _split_multi_waits(nc)
    return nc


_cache = {}


def _get_nc(n_steps):
    if n_steps not in _cache:
        _cache[n_steps] = build_nc(n_steps)
    return _cache[n_steps]


def _prep_inputs(x, W_ih0, W_hh0, b_ih0, b_hh0, W_ih1, W_hh1, b_ih1, b_hh1,
                 W_fc, b_fc, n_steps=T):
    nch = n_steps // CH

    def wT(w):  # [out, in] -> [in, out] contiguous bf16
        return np.ascontiguousarray(w.T).astype(F16)

    wg0 = np.zeros((KG * 128, H), np.float32)
    wg0[0:I, :] = W_ih0.T
    wg1 = np.zeros((KG * 128, H), np.float32)
    wg1[KX * 128:KX * 128 + H, :] = W_ih1.T

    def wg_pack(wg):  # [KG*128, H] -> [128, KG*H]
        return np.ascontiguousarray(
            wg.reshape(KG, 128, H).transpose(1, 0, 2).reshape(128, KG * H)
        ).astype(F16)

    def wh_pack(whh):  # W_hh [H,H] -> lhsT chunks [128, KH*H]
        t = whh.T.reshape(KH, 128, H).transpose(1, 0, 2)
        return np.ascontiguousarray(t.reshape(128, KH * H)).astype(F16)

    wfc = np.zeros((KH, 128, 8), np.float32)
    wfc[:, :, 0] = W_fc.reshape(KH, 128)
    wfc = np.ascontiguousarray(
        wfc.transpose(1, 0, 2).reshape(128, KH * 8)).astype(F16)

    b0 = (b_ih0 + b_hh0).reshape(KH, 128).astype(F16)
    b1 = (b_ih1 + b_hh1).reshape(KH, 128).astype(F16)
    neg = np.full((KH, 128), -60000.0, F16)
    ind = np.zeros((KH, KH * CH * GW), np.float32)
    for c in range(KH):
        ind[c, c * CH * GW:(c + 1) * CH * GW] = 1.0
    ind = ind.astype(F16)
    bfc = b_fc.reshape(1, 1).astype(np.float32)

    in_maps = []
    for c in range(NCORES):
        p = c % PAIRS
        role = c // PAIRS
        if role == 0:
            xs = x[p * NW:(p + 1) * NW, :n_steps]        # [16, t, I]
            # layout [kx][g][t][gw]: value x[g*8+b, t, kx*128+i]
            xt = xs.reshape(2, GW, n_steps, KX, 128)     # [g][b][t][kx][i]
            xt = xt.transpose(4, 3, 0, 2, 1)             # [i][kx][g][t][b]
            xt = np.ascontiguousarray(
                xt.reshape(128, KX * 2 * n_steps * GW)).astype(F16)
        else:
            xt = np.zeros((128, KX * 2 * n_steps * GW), F16)
        in_maps.append({
            "xt": xt,
            "wg": wg_pack(wg0 if role == 0 else wg1),
            "wh": wh_pack(W_hh0 if role == 0 else W_hh1),
            "bias": b0 if role == 0 else b1,
            "ebias": b0 if role == 0 else neg,
            "wfc": wfc,
            "bfc": bfc,
            "ind": ind,
        })
    return in_maps


def _postprocess(results, n_steps=T, se=None):
    lag = (SE if se is None else se) + 8
    nch = n_steps // CH
    out = np.zeros((B, n_steps, 1), np.float32)
    for p in range(PAIRS):
        y = results[PAIRS + p]["y"]                      # [niter, CH*NW]
        y = y[lag:lag + nch].reshape(nch, 2, CH, GW)     # [i][g][r][b]
        for g in range(2):
            blk = y[:, g, :, :].transpose(2, 0, 1).reshape(GW, n_steps)
            out[p * NW + g * GW:p * NW + (g + 1) * GW, :, 0] = blk
    return out


def kernel(x, W_ih0, W_hh0, b_ih0, b_hh0, W_ih1, W_hh1, b_ih1, b_hh1,
           W_fc, b_fc):
    args = [np.asarray(a, dtype=np.float32)
            for a in (x, W_ih0, W_hh0, b_ih0, b_hh0, W_ih1, W_hh1, b_ih1,
                      b_hh1, W_fc, b_fc)]
    nc = _get_nc(T)
    in_maps = _prep_inputs(*args)
    res = run_bass_kernel_spmd(nc, in_maps, core_ids=list(range(NCORES)))
    return _postprocess(res.results)

